# revision 39
# baseline (speedup 1.0000x reference)
"""MeanStdFilter kernel for 8 Trainium2 NeuronCores.

Semantics (matches the sequential-Welford reference with M=0, S=S_in, n=0):
    S1[f] = sum_b x[b, f]            (global, over all 32768 rows)
    S2[f] = sum_b x[b, f]^2
    mean  = S1 / N
    M2    = S2 - S1^2 / N + S_in     (Welford M2 started from buffer S)
    var   = M2 / (N - 1)             (N = 32768 > 1)
    out   = (x - mean) / (sqrt(var) + 1e-5)
The input running-mean buffer M is overwritten by the first Welford step in
the reference, so it never affects the output.

Design (v6; baseline traces showed DVE 123us busy / 205us total):
  - x sharded 4096 rows/core, streamed in fp32 (1 MB two-tile load DMAs,
    ~328 GB/s) and kept resident as BF16 (ACT casts, DVE squares -> bf16,
    PE bf16 ones-matmuls accumulate S1/S2 in PSUM). Phase A engines all
    run under the load DMA.
  - 8KB ncfw AllReduce of raw (S1,S2). Tried and rejected: a
    remote_dma_broadcast all-gather (broken in this environment: multi-ms
    stalls + corrupt slots - the fake_nrt shim lacks working cross-core
    SDMA routing), bf16 AR payload (no measurable win over the ~+-15us
    ncfw jitter), scalar-engine HWDGE queue (NEFF load rejection), SWDGE
    stores, and two-tile store DMAs (store BW regressed).
  - Broadcast-redundant finalize: ONE post-AR DMA broadcasts (S1|S2) to
    all partitions; each computes per-feature -mean and rstd redundantly
    (rstd via Abs_reciprocal_sqrt: the reference's +eps on std shifts the
    result ~1e-5 relative, far below bf16 resolution).
  - Phase C normalizes the bf16 copy with two-tile (FD=2048) in-place DVE
    ops (2x packed mode; halves per-op DRAIN/sem overhead), then per-tile
    ACT upcasts each immediately followed by its store (single-queue
    per-tile 512KB stores measured fastest, ~285-300 GB/s). bf16 keeps
    median rel err ~2e-3, well under the 2e-2 gate (the fp32 baseline's
    max rel err is already 3.3e-2 from summation-order noise).
  NOTE on measurement: the ncfw AR exec jitters 10-28us run-to-run, so
  single-run timings vary ~+-15us; best observed 160us, typical ~175-185.
"""

import functools

import numpy as np

import concourse.bacc as bacc
import concourse.tile as tile
from concourse import mybir
from concourse.bass_utils import run_bass_kernel_spmd

NCORES = 8
B, F = 32768, 1024
ROWS = B // NCORES  # 4096 rows per core
P = 128
NT = ROWS // P  # 32 row-tiles of [128, 1024] per core
EPS = 1e-5
FP32 = mybir.dt.float32
BF16 = mybir.dt.bfloat16
AF = mybir.ActivationFunctionType
ALU = mybir.AluOpType


def build_kernel():
    nc = bacc.Bacc(
        "TRN2", target_bir_lowering=False, debug=False, num_devices=NCORES
    )
    x = nc.declare_dram_parameter("x", [ROWS, F], FP32, isOutput=False)
    s_in = nc.declare_dram_parameter("S", [1, F], FP32, isOutput=False)
    out = nc.declare_dram_parameter("out", [ROWS, F], FP32, isOutput=True)

    x_t = x[:].rearrange("(n p) f -> n p f", p=P)
    x_t2 = x[:].rearrange("(c n p) f -> c p n f", n=2, p=P)
    out_t = out[:].rearrange("(n p) f -> n p f", p=P)
    out_t2 = out[:].rearrange("(c n p) f -> c p n f", n=2, p=P)
    groups = [list(range(NCORES))]

    with tile.TileContext(nc) as tc:
        with (
            tc.tile_pool(name="xf", bufs=6) as xfpool,
            tc.tile_pool(name="xb", bufs=1) as xbpool,
            tc.tile_pool(name="sq", bufs=3) as sqpool,
            tc.tile_pool(name="o32", bufs=8) as opool,
            tc.tile_pool(name="stats", bufs=1) as stats,
            tc.tile_pool(name="psum", bufs=1, space="PSUM") as psum,
            tc.tile_pool(name="dram", bufs=1, space="DRAM") as dram,
        ):
            # Resident bf16 shard: 4 chunks x [128, 8, 1024] (16 KB/part each).
            xb = [
                xbpool.tile([P, 8, F], BF16, tag=f"xb{c}", name=f"xb{c}")
                for c in range(4)
            ]

            def xtile(t):
                return xb[t // 8][:, t % 8, :]

            ones = stats.tile([P, 1], BF16)
            nc.vector.memset(ones, 1.0)

            # One PSUM bank per 512-wide half (4 banks total).
            ps1 = [psum.tile([1, 512], FP32, tag=f"ps1_{h}", name=f"ps1_{h}") for h in range(2)]
            ps2 = [psum.tile([1, 512], FP32, tag=f"ps2_{h}", name=f"ps2_{h}") for h in range(2)]

            # ---- Phase A: stream fp32 tiles (1 MB two-tile DMAs halve the
            # per-DMA fixed costs), cast to bf16, square, PE sums.
            prewarm = stats.tile([P, 8], FP32)
            for c in range(NT // 2):
                t0, t1 = 2 * c, 2 * c + 1
                xf2 = xfpool.tile([P, 2, F], FP32, tag="xf")
                nc.sync.dma_start(out=xf2[:], in_=x_t2[c])
                # Two-tile compute ops (FD=2048) halve per-op DRAIN/sem
                # overhead on ACT and DVE.
                xb2 = xb[t0 // 8][:, t0 % 8 : t0 % 8 + 2, :]
                nc.scalar.activation(xb2, xf2, AF.Copy)  # fp32 -> bf16 cast
                sq = sqpool.tile([P, 2, F], BF16, tag="sq")
                nc.vector.tensor_tensor(sq[:], xf2, xf2, ALU.mult)  # x^2
                for t in (t0, t1):
                    for h in range(2):
                        hs = slice(h * 512, (h + 1) * 512)
                        nc.tensor.matmul(
                            ps1[h][:],
                            lhsT=ones[:],
                            rhs=xtile(t)[:, hs],
                            start=(t == 0),
                            stop=(t == NT - 1),
                        )
                        nc.tensor.matmul(
                            ps2[h][:],
                            lhsT=ones[:],
                            rhs=sq[:, t % 2, hs],
                            start=(t == 0),
                            stop=(t == NT - 1),
                        )
                if c == 0:
                    # Pre-load the ACT rsqrt LUT so finalize doesn't pay the
                    # ~1.3us ACT_TABLE_LOAD on the critical path. Also kick
                    # the S broadcast load (finalize adds it to M2).
                    nc.vector.memset(prewarm, 1.0)
                    nc.scalar.activation(
                        prewarm, prewarm, AF.Abs_reciprocal_sqrt
                    )
                    sinb = stats.tile([P, F], FP32)
                    nc.sync.dma_start(
                        out=sinb[:], in_=s_in[:].to_broadcast([P, F])
                    )

            # Pack (S1, S2) f-major into one [1, 2048] staging tile for the
            # AR. Copies split across ACT and DVE so they drain in parallel.
            cc_stage = stats.tile([1, 2 * F], FP32)
            for h in range(2):
                nc.scalar.copy(cc_stage[:, h * 512 : (h + 1) * 512], ps1[h][:])
                nc.vector.tensor_copy(
                    cc_stage[:, F + h * 512 : F + (h + 1) * 512], ps2[h][:]
                )

            cc_in = dram.tile([1, 2 * F], FP32)
            cc_out = dram.tile([1, 2 * F], FP32)
            nc.sync.dma_start(out=cc_in[:], in_=cc_stage[:])
            nc.gpsimd.collective_compute(
                "AllReduce",
                ALU.add,
                replica_groups=groups,
                ins=[cc_in[:].opt()],
                outs=[cc_out[:].opt()],
            )

            # ---- Broadcast-redundant finalize: ONE post-AR DMA broadcasts
            # (S1|S2) [1, 2048] to all 128 partitions; every partition then
            # computes the full per-feature -mean/rstd redundantly. Avoids
            # the packed-finalize + DRAM-roundtrip + re-broadcast chain
            # (3 serial DMA receipts -> 1).
            s12_b = stats.tile([P, 2 * F], FP32)
            nc.sync.dma_start(out=s12_b[:], in_=cc_out[:].to_broadcast([P, 2 * F]))
            s1_b = s12_b[:, 0:F]
            s2_b = s12_b[:, F : 2 * F]
            nmean_b = stats.tile([P, F], FP32)
            nc.scalar.activation(nmean_b, s1_b, AF.Copy, scale=-1.0 / B)
            nc.vector.tensor_tensor(s1_b, s1_b, nmean_b, ALU.mult)  # -S1^2/N
            nc.vector.tensor_tensor(s2_b, s2_b, s1_b, ALU.add)  # M2
            nc.vector.tensor_tensor(s2_b, s2_b, sinb[:], ALU.add)  # + S_in
            rstd_f = stats.tile([P, F], FP32)
            nc.scalar.activation(
                rstd_f, s2_b, AF.Abs_reciprocal_sqrt, scale=1.0 / (B - 1)
            )
            rstd_b = stats.tile([P, F], BF16)
            nmr_b = stats.tile([P, F], BF16)
            nc.vector.tensor_copy(rstd_b[:], rstd_f)  # -> bf16
            nc.vector.tensor_tensor(nmr_b[:], nmean_b, rstd_f, ALU.mult)
            # Stride-0 middle-dim views feed the two-tile phase C ops
            # without materializing duplicate lanes.
            rstd_b2 = rstd_b[:, None, :].to_broadcast([P, 2, F])
            nmr_b2 = nmr_b[:, None, :].to_broadcast([P, 2, F])

            # ---- Phase C: normalize bf16 copy in place (2x-mode DVE ops),
            # cast back to fp32 on ACT, store.
            for c in range(NT // 2):
                t0 = 2 * c
                xb2 = xb[t0 // 8][:, t0 % 8 : t0 % 8 + 2, :]
                nc.vector.tensor_tensor(xb2, xb2, rstd_b2, ALU.mult)
                nc.vector.tensor_tensor(xb2, xb2, nmr_b2, ALU.add)
                # Per-tile casts + stores keep the store stream smooth (a
                # single 2-tile cast before paired stores measured ~20 GB/s
                # slower stores).
                for k in range(2):
                    o32 = opool.tile([P, F], FP32, tag="o32")
                    nc.scalar.activation(o32, xb2[:, k, :], AF.Copy)
                    nc.sync.dma_start(out=out_t[t0 + k], in_=o32[:])

    nc.finalize()
    return nc


@functools.cache
def _get_nc():
    return build_kernel()


def kernel(x, M, S, _trace=False, _trace_kwargs=None):
    del M  # overwritten by the first Welford step in the reference
    x = np.ascontiguousarray(x, dtype=np.float32)
    S = np.ascontiguousarray(S, dtype=np.float32).reshape(1, F)
    nc = _get_nc()
    in_maps = [
        {"x": x[i * ROWS : (i + 1) * ROWS], "S": S} for i in range(NCORES)
    ]
    res = run_bass_kernel_spmd(
        nc,
        in_maps,
        core_ids=list(range(NCORES)),
        trace=_trace,
        **(_trace_kwargs or {}),
    )
    out = np.concatenate([res.results[i]["out"] for i in range(NCORES)], axis=0)
    if _trace:
        return out, res
    return out
